# revision 14
# baseline (speedup 1.0000x reference)
# MoE routing hop (DNA) on 8 TRN2 NeuronCores — expert-parallel Bass/Tile kernel.
#
# Shapes (hardcoded): T=4096 tokens, D=1024, E=16 experts, DFF=1024, topk=2, capacity=512.
#
# Sharding: expert-parallel (2 experts/core). Router runs on each core's token
# block; logits are AllGathered (tiny) so every core has the full routing
# picture. Each core dispatches ALL masked tokens of its 2 experts (padded
# capacity CP=640 >= max n_e whp), runs the FFN in bf16, and the weighted
# outputs are AllGathered (split per local expert so the first AG overlaps the
# second expert's FFN); each core assembles its own 512-token output block by
# gathering each token's <=2 expert rows.
#
# v2: the per-expert gather index list (slot -> token id) is built with
# one-hot matmuls on the PE (two tiny matmuls decompose slot = 16*c + m) —
# no dma_scatter_add (the v1 scatter cost ~600us of latency-bound 256B
# CCE descriptors).
#
# Self-contained: no imports from /root/problem, everything hardcoded.
import sys

if "/opt/trn_rl_repo" not in sys.path:
    sys.path.insert(0, "/opt/trn_rl_repo")

import numpy as np

T, D, E, DFF = 4096, 1024, 16, 1024
TOPK, C = 2, 512
NCORES = 8
TB = T // NCORES        # 512 tokens per core block
EB = E // NCORES        # 2 experts per core
CP = 640                # padded per-expert capacity (n_e ~ 512 +- 21, 6 sigma)
CP0 = 512               # dispatch gather split (multiples of 128)
CP1 = CP - CP0
NC40 = CP // 16         # 40 wrapped-idx columns
J = T // 128            # 32 j-chunks over full T (token t = j*128 + p)
JB = TB // 128          # 4 j-chunks per block
NBIS = 31               # bisection iterations over [-8, 8] (fp32-exact cut)
SPLIT_AG = False        # split output AllGather per expert (overlap w/ FFN)

_cache = {}


def _build_program(local_only=False):
    import concourse.bass as bass
    import concourse.mybir as mybir
    import concourse.tile as tile
    from concourse import bacc

    f32 = mybir.dt.float32
    bf16 = mybir.dt.bfloat16
    i16 = mybir.dt.int16
    i32 = mybir.dt.int32
    Alu = mybir.AluOpType
    Act = mybir.ActivationFunctionType

    nc = bacc.Bacc("TRN2", target_bir_lowering=False, debug=False, num_devices=NCORES)

    def ap_ins0(a, count, at=1):
        # insert a step-0 (broadcast) dim into an AP at position `at`
        dims = [list(d) for d in a.ap]
        dims.insert(at, [0, count])
        return bass.AP(a.tensor, a.offset, dims)

    def ap_swap_free(a):
        # [128, A, B] -> dims reordered so B is outer, A inner (for reducing A)
        dims = [list(d) for d in a.ap]
        assert len(dims) == 3
        return bass.AP(a.tensor, a.offset, [dims[0], dims[2], dims[1]])

    # ---------------- I/O ----------------
    hT_blk = nc.dram_tensor("hT_blk", [D, TB], f32, kind="ExternalInput")
    h_blk = nc.dram_tensor("h_blk", [TB, D], f32, kind="ExternalInput")
    h_bf = nc.dram_tensor("h_bf", [T, D], bf16, kind="ExternalInput")
    Wr_t = nc.dram_tensor("Wr", [D, E], f32, kind="ExternalInput")
    W1b = nc.dram_tensor("W1b", [EB, D, DFF], bf16, kind="ExternalInput")
    W2b = nc.dram_tensor("W2b", [EB, DFF, D], bf16, kind="ExternalInput")
    TRI = nc.dram_tensor("TRI", [128, 128], f32, kind="ExternalInput")     # p'<=p
    TRIX = nc.dram_tensor("TRIX", [J, J], f32, kind="ExternalInput")       # j'<j
    ONE1 = nc.dram_tensor("ONE1", [1, 128], f32, kind="ExternalInput")
    EIOTA = nc.dram_tensor("EIOTA", [128, E], f32, kind="ExternalInput")   # 0..15 per row
    ESEL = nc.dram_tensor("ESEL", [128, 2 * E], f32, kind="ExternalInput")  # per-core onehots
    JSEL = nc.dram_tensor("JSEL", [128, JB * J], f32, kind="ExternalInput")  # per-core col onehots
    VALT = nc.dram_tensor("VALT", [128, J], f32, kind="ExternalInput")     # t = j*128+p
    CIO40 = nc.dram_tensor("CIO40", [128, NC40], f32, kind="ExternalInput")  # 0..39 per row
    MI16 = nc.dram_tensor("MI16", [128, 16], f32, kind="ExternalInput")    # 0..15 per row
    REPM = nc.dram_tensor("REPM", [16, 128], f32, kind="ExternalInput")    # [m, m'] = m'%16==m
    out_t = nc.dram_tensor("out", [TB, D], f32, kind="ExternalOutput")

    with tile.TileContext(nc) as tc:
        import contextlib

        with contextlib.ExitStack() as top:
            # ---------------- pools ----------------
            main = top.enter_context(tc.tile_pool(name="main", bufs=1))
            psS = top.enter_context(tc.tile_pool(name="psS", bufs=1, space="PSUM"))
            psP = top.enter_context(tc.tile_pool(name="psP", bufs=1, space="PSUM"))
            psDa = top.enter_context(tc.tile_pool(name="psDa", bufs=1, space="PSUM"))
            psDb = top.enter_context(tc.tile_pool(name="psDb", bufs=1, space="PSUM"))
            dram = top.enter_context(tc.tile_pool(name="dram", bufs=1, space="DRAM"))

            # collective buffers (internal DRAM)
            blob_in = dram.tile([TB, E], f32, name="blob_in")
            blob_out = dram.tile([T, E], f32, name="blob_out",
                                 addr_space="Local" if local_only else "Shared")
            if SPLIT_AG:
                agins = [dram.tile([CP, D], bf16, name=f"agin{i}") for i in range(EB)]
                agouts = [dram.tile([NCORES * CP, D], bf16, name=f"agout{i}",
                                    addr_space="Local" if local_only else "Shared")
                          for i in range(EB)]
            else:
                agin = dram.tile([EB * CP, D], bf16, name="agin")
                agout = dram.tile([E * CP, D], bf16, name="agout",
                                  addr_space="Local" if local_only else "Shared")

            # =========== Phase R: router (own block) + AllGather logits ===========
            with tc.tile_pool(name="router", bufs=1) as rp:
                hTsb = rp.tile([128, D // 128, TB], f32, name="hTsb")
                wrsb = rp.tile([128, D // 128, E], f32, name="wrsb")
                lgb = rp.tile([128, JB, E], f32, name="lgb")
                nc.sync.dma_start(hTsb[:], hT_blk[:].rearrange("(dt p) t -> p dt t", p=128))
                nc.sync.dma_start(wrsb[:], Wr_t[:].rearrange("(dt p) e -> p dt e", p=128))
                for tt in range(JB):
                    pslg = psS.tile([128, E], f32, name="pslg", tag="pslg")
                    for dt in range(D // 128):
                        nc.tensor.matmul(
                            pslg[:],
                            hTsb[:, dt, tt * 128:(tt + 1) * 128],
                            wrsb[:, dt, :],
                            start=(dt == 0),
                            stop=(dt == D // 128 - 1),
                        )
                    nc.vector.tensor_copy(lgb[:, tt, :], pslg[:])
                nc.sync.dma_start(
                    blob_in[:].rearrange("(tt p) e -> p tt e", p=128), lgb[:]
                )
            if local_only:
                for r in range(NCORES):
                    nc.sync.dma_start(blob_out[:].rearrange("(r t) e -> r t e", r=NCORES)[r], blob_in[:])
            else:
                nc.gpsimd.collective_compute(
                    "AllGather",
                    Alu.bypass,
                    replica_groups=[list(range(NCORES))],
                    ins=[blob_in[:]],
                    outs=[blob_out[:]],
                )

            # =========== Weight + h prefetch (ACT HWDGE ring, overlaps AG) ===========
            wp = top.enter_context(tc.tile_pool(name="wpool", bufs=1))
            w1sb = {}
            w2sb = {}
            for i in range(EB):
                for dt in range(D // 128):
                    w1sb[i, dt] = wp.tile([128, DFF], bf16, name=f"w1_{i}_{dt}")
                    nc.scalar.dma_start(w1sb[i, dt][:], W1b[i, dt * 128:(dt + 1) * 128, :])
                for ft in range(DFF // 128):
                    w2sb[i, ft] = wp.tile([128, D], bf16, name=f"w2_{i}_{ft}")
                    nc.scalar.dma_start(w2sb[i, ft][:], W2b[i, ft * 128:(ft + 1) * 128, :])
            hsb = main.tile([128, JB, D], f32, name="hsb")
            nc.scalar.dma_start(hsb[:], h_blk[:].rearrange("(j p) d -> p j d", p=128))

            # ---------------- constants in ----------------
            trisb = main.tile([128, 128], f32, name="trisb")
            trixsb = main.tile([J, J], f32, name="trixsb")
            one1sb = main.tile([1, 128], f32, name="one1sb")
            eiota = main.tile([128, E], f32, name="eiota")
            esel = main.tile([128, 2 * E], f32, name="esel")
            jsel = main.tile([128, JB * J], f32, name="jsel")
            valt = main.tile([128, J], f32, name="valt")
            cio40 = main.tile([128, NC40], f32, name="cio40")
            mi16 = main.tile([128, 16], f32, name="mi16")
            repsb = main.tile([16, 128], f32, name="repsb")
            ones128 = main.tile([128, 128], f32, name="ones128")
            nc.sync.dma_start(trisb[:], TRI[:])
            nc.sync.dma_start(trixsb[:], TRIX[:])
            nc.sync.dma_start(one1sb[:], ONE1[:])
            nc.sync.dma_start(eiota[:], EIOTA[:])
            nc.sync.dma_start(esel[:], ESEL[:])
            nc.sync.dma_start(jsel[:], JSEL[:])
            nc.sync.dma_start(valt[:], VALT[:])
            nc.sync.dma_start(cio40[:], CIO40[:])
            nc.sync.dma_start(mi16[:], MI16[:])
            nc.sync.dma_start(repsb[:], REPM[:])
            nc.vector.memset(ones128[:], 1.0)
            c16 = main.tile([128, NC40], f32, name="c16")
            c16p = main.tile([128, NC40], f32, name="c16p")
            nc.vector.tensor_scalar(c16[:], cio40[:], 16.0, None, op0=Alu.mult)
            nc.vector.tensor_scalar(c16p[:], c16[:], 16.0, None, op0=Alu.add)

            # long-lived sbuf tiles
            lg = main.tile([128, J, E], f32, name="lg")
            mask = main.tile([128, J, E], f32, name="mask")
            lgm = main.tile([128, J, E], f32, name="lgm")
            probs = main.tile([128, J, E], f32, name="probs")
            pos = main.tile([128, J, E], f32, name="pos")
            cw = main.tile([128, J, E], f32, name="cw")
            tmp_jes = [main.tile([128, J, E], f32, name=f"tmp_je{i}") for i in range(2)]
            m1 = main.tile([128, J], f32, name="m1")
            m2 = main.tile([128, J], f32, name="m2")
            rs = main.tile([128, J], f32, name="rs")
            rho = main.tile([128, J], f32, name="rho")
            bp32 = main.tile([J, E], f32, name="bp32")
            bprow = main.tile([1, J * E], f32, name="bprow")

            nc.sync.dma_start(lg[:], blob_out[:].rearrange("(j p) e -> p j e", p=128))

            # =========== Phase M: routing stats (token layout, replicated) ===========
            nc.vector.tensor_reduce(m1[:], lg[:], axis=mybir.AxisListType.X, op=Alu.max)
            # masked = lg - 1e30*(lg == m1)
            nc.vector.tensor_tensor(
                out=tmp_jes[0][:], in0=lg[:], in1=ap_ins0(m1[:], E, at=2), op=Alu.is_equal
            )
            nc.vector.tensor_scalar(tmp_jes[0][:], tmp_jes[0][:], -1e30, None, op0=Alu.mult)
            nc.vector.tensor_tensor(
                out=tmp_jes[1][:], in0=lg[:], in1=tmp_jes[0][:], op=Alu.add
            )
            nc.vector.tensor_reduce(m2[:], tmp_jes[1][:], axis=mybir.AxisListType.X, op=Alu.max)
            nc.vector.tensor_tensor(
                out=mask[:], in0=lg[:], in1=ap_ins0(m2[:], E, at=2), op=Alu.is_ge
            )
            # lgm = mask ? lg : -1e30  ==  (mask*1e30 - 1e30) + lg*mask  (no absorption)
            nc.vector.tensor_scalar(lgm[:], mask[:], 1e30, -1e30, op0=Alu.mult, op1=Alu.add)
            nc.vector.tensor_tensor(out=tmp_jes[1][:], in0=lg[:], in1=mask[:], op=Alu.mult)
            nc.vector.tensor_tensor(out=lgm[:], in0=lgm[:], in1=tmp_jes[1][:], op=Alu.add)
            # probs (unnormalized-exp trick; |logits| small)
            ex = tmp_jes[0]
            nc.scalar.activation(ex[:], lg[:], Act.Exp)
            nc.vector.tensor_reduce(rs[:], ex[:], axis=mybir.AxisListType.X, op=Alu.add)
            nc.vector.reciprocal(rs[:], rs[:])
            nc.vector.tensor_tensor(
                out=probs[:], in0=ex[:], in1=ap_ins0(rs[:], E, at=2), op=Alu.mult
            )

            # =========== Phase P: pos = per-expert inclusive cumsum over t ===========
            pspos = psP.tile([128, J * E], f32, name="pspos", tag="pspos")
            for j in range(J):
                nc.tensor.matmul(
                    pspos[:, j * E:(j + 1) * E], trisb[:], mask[:, j, :],
                    start=True, stop=True,
                )
            nc.vector.tensor_copy(pos[:], pspos[:].rearrange("p (j e) -> p j e", e=E))
            nc.sync.dma_start(bp32[:], pos[127:128, :, :])
            psbp = psS.tile([J, E], f32, name="psbp", tag="pslg")
            nc.tensor.matmul(psbp[:], trixsb[:], bp32[:], start=True, stop=True)
            nc.vector.tensor_copy(bp32[:], psbp[:])
            nc.sync.dma_start(
                bprow[0:1, :].rearrange("x (j e) -> x j e", e=E), bp32[:]
            )
            psbc = psP.tile([128, J * E], f32, name="psbc", tag="pspos")
            for j in range(J):
                nc.tensor.matmul(
                    psbc[:, j * E:(j + 1) * E], one1sb[:], bprow[0:1, j * E:(j + 1) * E],
                    start=True, stop=True,
                )
            nc.vector.tensor_tensor(
                out=pos[:], in0=pos[:], in1=psbc[:].rearrange("p (j e) -> p j e", e=E),
                op=Alu.add,
            )

            # =========== Phase X: dispatch — build gather idx list via one-hot matmuls ===========
            # slot s = 16*c + m (c in [0,40), m in [0,16)); idxs layout for
            # dma_gather is [p=s%16 (x8 replicas), col=s//16].
            xTa = [main.tile([128, D // 128, CP0], bf16, name=f"xTa{i}") for i in range(EB)]
            xTb = [main.tile([128, D // 128, CP1], bf16, name=f"xTb{i}") for i in range(EB)]
            idxsA = [main.tile([128, CP0 // 16], i16, name=f"idxsA{i}") for i in range(EB)]
            idxsB = [main.tile([128, CP1 // 16], i16, name=f"idxsB{i}") for i in range(EB)]
            for i in range(EB):
                my_mask = main.tile([128, J], f32, name=f"my_mask{i}")
                my_pos = main.tile([128, J], f32, name=f"my_pos{i}")
                sel = esel[:, i * E:(i + 1) * E]
                nc.vector.tensor_tensor(
                    out=tmp_jes[1][:], in0=mask[:], in1=ap_ins0(sel, J), op=Alu.mult
                )
                nc.vector.tensor_reduce(
                    my_mask[:], tmp_jes[1][:], axis=mybir.AxisListType.X, op=Alu.add
                )
                nc.vector.tensor_tensor(
                    out=tmp_jes[1][:], in0=pos[:], in1=ap_ins0(sel, J), op=Alu.mult
                )
                nc.vector.tensor_reduce(
                    my_pos[:], tmp_jes[1][:], axis=mybir.AxisListType.X, op=Alu.add
                )
                # idxm = mask ? pos-1 : 1e6   via (pos-1-1e6)*mask + 1e6
                idxm = main.tile([128, J], f32, name=f"idxm{i}")
                nc.vector.tensor_scalar(idxm[:], my_pos[:], -1.0 - 1e6, None, op0=Alu.add)
                nc.vector.tensor_tensor(out=idxm[:], in0=idxm[:], in1=my_mask[:], op=Alu.mult)
                nc.vector.tensor_scalar(idxm[:], idxm[:], 1e6, None, op0=Alu.add)
                # float split: ohd[c] = (idxm>=16c) - (idxm>=16c+16); div = sum c*ohd
                modv = main.tile([128, J], f32, name=f"modv{i}")
                ohd = main.tile([128, J, NC40], f32, name=f"ohd{i}")
                oht = main.tile([128, J, NC40], f32, name=f"oht{i}")
                ohm = main.tile([128, J, 16], f32, name=f"ohm{i}")
                nc.vector.tensor_tensor(
                    out=ohd[:], in0=ap_ins0(idxm[:], NC40, at=2),
                    in1=ap_ins0(c16[:], J, at=1), op=Alu.is_ge,
                )
                nc.vector.tensor_tensor(
                    out=oht[:], in0=ap_ins0(idxm[:], NC40, at=2),
                    in1=ap_ins0(c16p[:], J, at=1), op=Alu.is_ge,
                )
                nc.vector.tensor_tensor(out=ohd[:], in0=ohd[:], in1=oht[:], op=Alu.subtract)
                nc.vector.tensor_tensor(
                    out=oht[:], in0=ohd[:], in1=ap_ins0(cio40[:], J, at=1), op=Alu.mult
                )
                nc.vector.tensor_reduce(modv[:], oht[:], axis=mybir.AxisListType.X, op=Alu.add)
                # modv currently = div; mod = idxm - 16*div
                nc.vector.tensor_scalar(modv[:], modv[:], -16.0, None, op0=Alu.mult)
                nc.vector.tensor_tensor(out=modv[:], in0=modv[:], in1=idxm[:], op=Alu.add)
                nc.vector.tensor_tensor(
                    out=ohm[:], in0=ap_ins0(modv[:], 16, at=2),
                    in1=ap_ins0(mi16[:], J, at=1), op=Alu.is_equal,
                )
                nc.vector.tensor_tensor(
                    out=ohd[:], in0=ohd[:], in1=ap_ins0(valt[:], NC40, at=2), op=Alu.mult
                )
                # psA[m, c] = sum_t ohm * ohd*val
                psA = psDa.tile([16, NC40], f32, name="psA", tag="psA")
                for j in range(J):
                    nc.tensor.matmul(
                        psA[:], ohm[:, j, :], ohd[:, j, :],
                        start=(j == 0), stop=(j == J - 1),
                    )
                sb16 = main.tile([16, NC40], f32, name=f"sb16_{i}")
                nc.vector.tensor_copy(sb16[:], psA[:])
                # replicate 16 -> 128 partitions: psB[m', c] = sum_m REP[m, m'] sb16[m, c]
                psB = psDb.tile([128, NC40], f32, name="psB", tag="psB")
                nc.tensor.matmul(psB[:], repsb[:], sb16[:], start=True, stop=True)
                nc.vector.tensor_copy(idxsA[i][:], psB[:, 0:CP0 // 16])
                nc.vector.tensor_copy(idxsB[i][:], psB[:, CP0 // 16:NC40])
                # gather xT: [128, 8, CP] bf16 (d = dt*128 + p), split 512+128
                nc.gpsimd.dma_gather(
                    out_ap=xTa[i][:], in_ap=h_bf[:], idxs_ap=idxsA[i][:],
                    num_idxs=CP0, num_idxs_reg=CP0, elem_size=D, transpose=True,
                )
                nc.gpsimd.dma_gather(
                    out_ap=xTb[i][:], in_ap=h_bf[:], idxs_ap=idxsB[i][:],
                    num_idxs=CP1, num_idxs_reg=CP1, elem_size=D, transpose=True,
                )

            # =========== Phase F: FFN (bf16), expert-serial so AG#0 overlaps e1 ===========
            hidT = [main.tile([128, DFF // 128, CP], bf16, name=f"hidT{i}") for i in range(EB)]
            with tc.tile_pool(name="psF", bufs=2, space="PSUM") as psF, tc.tile_pool(
                name="yspool", bufs=3
            ) as ysp:
                for i in range(EB):
                    for ft in range(DFF // 128):
                        ps1 = psF.tile([128, CP], f32, name="ps1", tag="psf")
                        for dt in range(D // 128):
                            st = dt == 0
                            sp = dt == D // 128 - 1
                            nc.tensor.matmul(
                                ps1[:, 0:CP0],
                                w1sb[i, dt][:, ft * 128:(ft + 1) * 128],
                                xTa[i][:, dt, :],
                                start=st, stop=sp,
                            )
                            nc.tensor.matmul(
                                ps1[:, CP0:CP],
                                w1sb[i, dt][:, ft * 128:(ft + 1) * 128],
                                xTb[i][:, dt, :],
                                start=st, stop=sp,
                            )
                        nc.scalar.activation(hidT[i][:, ft, :], ps1[:], Act.Gelu_apprx_tanh)

                    # layer 2 + store to agin_i
                    for ct in range(CP // 128):
                        ps2 = psF.tile([128, D], f32, name="ps2", tag="psf")
                        for ft in range(DFF // 128):
                            st = ft == 0
                            sp = ft == DFF // 128 - 1
                            nc.tensor.matmul(
                                ps2[:, 0:512],
                                hidT[i][:, ft, ct * 128:(ct + 1) * 128],
                                w2sb[i, ft][:, 0:512],
                                start=st, stop=sp,
                            )
                            nc.tensor.matmul(
                                ps2[:, 512:D],
                                hidT[i][:, ft, ct * 128:(ct + 1) * 128],
                                w2sb[i, ft][:, 512:D],
                                start=st, stop=sp,
                            )
                        ys = ysp.tile([128, D], bf16, name="ys", tag="ys")
                        nc.scalar.activation(ys[:], ps2[:], Act.Copy)
                        if SPLIT_AG:
                            nc.scalar.dma_start(
                                agins[i][:].rearrange("(s p) d -> p s d", p=128)[:, ct:ct + 1, :],
                                ys[:],
                            )
                        else:
                            s = i * (CP // 128) + ct
                            nc.scalar.dma_start(
                                agin[:].rearrange("(s p) d -> p s d", p=128)[:, s:s + 1, :],
                                ys[:],
                            )
                    # AllGather this expert's outputs (AG#0 overlaps e1 FFN)
                    if SPLIT_AG:
                        if local_only:
                            for r in range(NCORES):
                                nc.sync.dma_start(
                                    agouts[i][:].rearrange("(r s) d -> r s d", r=NCORES)[r],
                                    agins[i][:],
                                )
                        else:
                            nc.gpsimd.collective_compute(
                                "AllGather",
                                Alu.bypass,
                                replica_groups=[list(range(NCORES))],
                                ins=[agins[i][:]],
                                outs=[agouts[i][:]],
                            )
                if not SPLIT_AG:
                    if local_only:
                        for r in range(NCORES):
                            nc.sync.dma_start(
                                agout[:].rearrange("(r s) d -> r s d", r=NCORES)[r],
                                agin[:],
                            )
                    else:
                        nc.gpsimd.collective_compute(
                            "AllGather",
                            Alu.bypass,
                            replica_groups=[list(range(NCORES))],
                            ins=[agin[:]],
                            outs=[agout[:]],
                        )

            # =========== Bisection for capacity threshold (DVE, overlaps FFN) ===========
            lo_t = main.tile([128, E], f32, name="lo_t")
            hi_t = main.tile([128, E], f32, name="hi_t")
            mid_t = main.tile([128, E], f32, name="mid_t")
            cntp = main.tile([128, E], f32, name="cntp")
            pred = main.tile([128, E], mybir.dt.uint8, name="pred")
            predn = main.tile([128, E], mybir.dt.uint8, name="predn")
            nc.vector.memset(lo_t[:], -8.0)
            nc.vector.memset(hi_t[:], 8.0)
            for it in range(NBIS):
                nc.vector.tensor_tensor(out=mid_t[:], in0=lo_t[:], in1=hi_t[:], op=Alu.add)
                nc.vector.tensor_scalar(mid_t[:], mid_t[:], 0.5, None, op0=Alu.mult)
                cmpm = tmp_jes[0]
                nc.vector.tensor_tensor(
                    out=cmpm[:], in0=lgm[:], in1=ap_ins0(mid_t[:], J), op=Alu.is_gt
                )
                nc.vector.tensor_reduce(
                    cntp[:], ap_swap_free(cmpm[:]), axis=mybir.AxisListType.X, op=Alu.add
                )
                pscnt = psS.tile([128, E], f32, name="pscnt", tag="pslg")
                nc.tensor.matmul(pscnt[:], ones128[:], cntp[:], start=True, stop=True)
                nc.vector.tensor_scalar(pred[:], pscnt[:], float(C), None, op0=Alu.is_gt)
                nc.vector.tensor_scalar(predn[:], pscnt[:], float(C), None, op0=Alu.is_le)
                nc.vector.copy_predicated(lo_t[:], pred[:], mid_t[:])
                nc.vector.copy_predicated(hi_t[:], predn[:], mid_t[:])

            # combine weights cw = probs * (lgm > hi)  (lgm=-1e30 for unmasked)
            nc.vector.tensor_tensor(
                out=cw[:], in0=lgm[:], in1=ap_ins0(hi_t[:], J), op=Alu.is_gt
            )
            nc.vector.tensor_tensor(out=cw[:], in0=cw[:], in1=probs[:], op=Alu.mult)
            nc.vector.tensor_reduce(rho[:], cw[:], axis=mybir.AxisListType.X, op=Alu.add)

            # =========== Phase G: per-token combine for my block ===========
            # e1/e2: the two masked expert ids per token; p1/p2 their pos
            e1 = main.tile([128, J], f32, name="e1")
            e2 = main.tile([128, J], f32, name="e2")
            p1 = main.tile([128, J], f32, name="p1")
            p2 = main.tile([128, J], f32, name="p2")
            w1f = main.tile([128, J], f32, name="w1f")
            w2f = main.tile([128, J], f32, name="w2f")
            emsk = tmp_jes[0]
            oh = tmp_jes[1]
            nc.vector.tensor_tensor(
                out=emsk[:], in0=mask[:], in1=ap_ins0(eiota[:], J), op=Alu.mult
            )
            nc.vector.tensor_scalar(
                oh[:], mask[:], -100000.0, 100000.0, op0=Alu.mult, op1=Alu.add
            )
            nc.vector.tensor_tensor(out=emsk[:], in0=emsk[:], in1=oh[:], op=Alu.add)
            nc.vector.tensor_reduce(e1[:], emsk[:], axis=mybir.AxisListType.X, op=Alu.min)
            nc.vector.tensor_tensor(
                out=oh[:], in0=ap_ins0(eiota[:], J), in1=ap_ins0(e1[:], E, at=2),
                op=Alu.is_equal,
            )
            nc.vector.tensor_scalar(oh[:], oh[:], 200000.0, None, op0=Alu.mult)
            nc.vector.tensor_tensor(out=emsk[:], in0=emsk[:], in1=oh[:], op=Alu.add)
            nc.vector.tensor_reduce(e2[:], emsk[:], axis=mybir.AxisListType.X, op=Alu.min)
            for (ei, pi, wif) in ((e1, p1, w1f), (e2, p2, w2f)):
                nc.vector.tensor_tensor(
                    out=oh[:], in0=ap_ins0(eiota[:], J), in1=ap_ins0(ei[:], E, at=2),
                    op=Alu.is_equal,
                )
                nc.vector.tensor_tensor(out=emsk[:], in0=oh[:], in1=cw[:], op=Alu.mult)
                nc.vector.tensor_reduce(wif[:], emsk[:], axis=mybir.AxisListType.X, op=Alu.add)
                nc.vector.tensor_tensor(out=oh[:], in0=oh[:], in1=pos[:], op=Alu.mult)
                nc.vector.tensor_reduce(pi[:], oh[:], axis=mybir.AxisListType.X, op=Alu.add)
                # slot = min(pos-1, CP-1)
                nc.vector.tensor_scalar(pi[:], pi[:], -1.0, float(CP - 1), op0=Alu.add, op1=Alu.min)
            if SPLIT_AG:
                # flat = (e>>1)*CP + slot into agout[e&1]; par = e&1 selects buffer
                par_f = [main.tile([128, J], f32, name=f"par_f{i}") for i in range(2)]
                eio2 = main.tile([128, 8], f32, name="eio2")
                oh8 = main.tile([128, J, 8], f32, name="oh8")
                nc.vector.tensor_scalar(eio2[:], eiota[:, 0:8], 2.0, 2.0, op0=Alu.mult, op1=Alu.add)
                for fi, (ei, pi) in enumerate(((e1, p1), (e2, p2))):
                    # ehal = floor(e/2) = sum_k 1[e >= 2k+2]; par = e - 2*ehal
                    ehalf = main.tile([128, J], f32, name=f"ehalf_{fi}")
                    nc.vector.tensor_tensor(
                        out=oh8[:], in0=ap_ins0(ei[:], 8, at=2),
                        in1=ap_ins0(eio2[:], J, at=1), op=Alu.is_ge,
                    )
                    nc.vector.tensor_reduce(ehalf[:], oh8[:], axis=mybir.AxisListType.X, op=Alu.add)
                    nc.vector.tensor_scalar(par_f[fi][:], ehalf[:], -2.0, None, op0=Alu.mult)
                    nc.vector.tensor_tensor(out=par_f[fi][:], in0=par_f[fi][:], in1=ei[:], op=Alu.add)
                    nc.vector.tensor_scalar(ehalf[:], ehalf[:], float(CP), None, op0=Alu.mult)
                    nc.vector.tensor_tensor(out=pi[:], in0=pi[:], in1=ehalf[:], op=Alu.add)
            else:
                # flat = e*CP + slot into single agout
                etmp = main.tile([128, J], f32, name="etmp")
                for (ei, pi) in ((e1, p1), (e2, p2)):
                    nc.vector.tensor_scalar(etmp[:], ei[:], float(CP), None, op0=Alu.mult)
                    nc.vector.tensor_tensor(out=pi[:], in0=pi[:], in1=etmp[:], op=Alu.add)

            # extract my block's columns via JSEL
            flat_blk = [main.tile([128, JB], f32, name=f"flat_blk{i}") for i in range(2)]
            par_blk = [main.tile([128, JB], f32, name=f"par_blk{i}") for i in range(2)]
            rho_blk = main.tile([128, JB], f32, name="rho_blk")
            wb = [main.tile([128, JB], f32, name=f"wb{i}") for i in range(2)]
            selmul = main.tile([128, J], f32, name="selmul")
            extracts = [(p1, flat_blk[0]), (p2, flat_blk[1]), (rho, rho_blk),
                        (w1f, wb[0]), (w2f, wb[1])]
            if SPLIT_AG:
                extracts += [(par_f[0], par_blk[0]), (par_f[1], par_blk[1])]
            for src, dst in extracts:
                for jb in range(JB):
                    nc.vector.tensor_tensor(
                        out=selmul[:], in0=src[:], in1=jsel[:, jb * J:(jb + 1) * J], op=Alu.mult
                    )
                    nc.vector.tensor_reduce(
                        dst[:, jb:jb + 1], selmul[:], axis=mybir.AxisListType.X, op=Alu.add
                    )
            if SPLIT_AG:
                # fA = flat + 1e6*par (skipped in agout0 when par=1); fB = flat + 1e6*(1-par)
                fidx = {}
                ftmp = main.tile([128, JB], f32, name="ftmp")
                for fi in range(2):
                    for buf in range(2):
                        fidx[fi, buf] = main.tile([128, JB], i32, name=f"fidx{fi}{buf}")
                        if buf == 0:
                            nc.vector.tensor_scalar(ftmp[:], par_blk[fi][:], 1e6, None, op0=Alu.mult)
                        else:
                            nc.vector.tensor_scalar(
                                ftmp[:], par_blk[fi][:], -1e6, 1e6, op0=Alu.mult, op1=Alu.add
                            )
                        nc.vector.tensor_tensor(out=ftmp[:], in0=ftmp[:], in1=flat_blk[fi][:], op=Alu.add)
                        nc.vector.tensor_copy(fidx[fi, buf][:], ftmp[:])
            else:
                flat_i32 = [main.tile([128, JB], i32, name=f"flat_i32{i}") for i in range(2)]
                nc.vector.tensor_copy(flat_i32[0][:], flat_blk[0][:])
                nc.vector.tensor_copy(flat_i32[1][:], flat_blk[1][:])
            wbb = [main.tile([128, JB], bf16, name=f"wbb{i}") for i in range(2)]
            nc.vector.tensor_copy(wbb[0][:], wb[0][:])
            nc.vector.tensor_copy(wbb[1][:], wb[1][:])

            with tc.tile_pool(name="fin", bufs=1) as fp:
                g1 = fp.tile([128, JB, D], bf16, name="g1")
                g2 = fp.tile([128, JB, D], bf16, name="g2")
                gt = fp.tile([128, JB, D], bf16, name="gt")
                if SPLIT_AG:
                    nc.vector.memset(g1[:], 0)
                    nc.vector.memset(g2[:], 0)
                    for gdst, fi in ((g1, 0), (g2, 1)):
                        for buf in range(2):
                            nc.gpsimd.indirect_dma_start(
                                out=gdst[:, :, :], out_offset=None, in_=agouts[buf][:],
                                in_offset=bass.IndirectOffsetOnAxis(
                                    ap=fidx[fi, buf][:], axis=0),
                                bounds_check=NCORES * CP - 1,
                                oob_is_err=False,
                                compute_op=Alu.add,
                            )
                else:
                    for jb in range(JB):
                        nc.gpsimd.indirect_dma_start(
                            out=g1[:, jb, :], out_offset=None, in_=agout[:],
                            in_offset=bass.IndirectOffsetOnAxis(
                                ap=flat_i32[0][:, jb:jb + 1], axis=0),
                        )
                        nc.gpsimd.indirect_dma_start(
                            out=g2[:, jb, :], out_offset=None, in_=agout[:],
                            in_offset=bass.IndirectOffsetOnAxis(
                                ap=flat_i32[1][:, jb:jb + 1], axis=0),
                        )
                # out = h*(1-rho) + w1*g1 + w2*g2
                omr = main.tile([128, JB], f32, name="omr")
                nc.vector.tensor_scalar(omr[:], rho_blk[:], -1.0, 1.0, op0=Alu.mult, op1=Alu.add)
                nc.vector.tensor_tensor(
                    out=hsb[:], in0=hsb[:], in1=ap_ins0(omr[:], D, at=2), op=Alu.mult
                )
                # gt = g1*w1 (bf16, 2x DVE), hsb += gt; gt = g2*w2, hsb += gt
                nc.vector.tensor_tensor(
                    out=gt[:], in0=g1[:], in1=ap_ins0(wbb[0][:], D, at=2), op=Alu.mult
                )
                nc.vector.tensor_tensor(out=hsb[:], in0=hsb[:], in1=gt[:], op=Alu.add)
                nc.vector.tensor_tensor(
                    out=gt[:], in0=g2[:], in1=ap_ins0(wbb[1][:], D, at=2), op=Alu.mult
                )
                nc.vector.tensor_tensor(out=hsb[:], in0=hsb[:], in1=gt[:], op=Alu.add)
                nc.sync.dma_start(out_t[:].rearrange("(j p) d -> p j d", p=128), hsb[:])

    nc.compile()
    return nc


def _prep_inputs(h, Wr, W1, W2):
    import ml_dtypes

    bf = ml_dtypes.bfloat16
    h = np.asarray(h, np.float32)
    Wr = np.asarray(Wr, np.float32)
    W1 = np.asarray(W1, np.float32)
    W2 = np.asarray(W2, np.float32)
    h_bf = h.astype(bf)
    TRI = np.triu(np.ones((128, 128), np.float32))          # [p', p] = p' <= p
    TRIX = np.triu(np.ones((J, J), np.float32), 1)          # [j', j] = j' < j
    ONE1 = np.ones((1, 128), np.float32)
    EIOTA = np.tile(np.arange(E, dtype=np.float32), (128, 1))
    VALT = (np.arange(128, dtype=np.float32)[:, None]
            + 128.0 * np.arange(J, dtype=np.float32)[None, :]).astype(np.float32)
    CIO40 = np.tile(np.arange(NC40, dtype=np.float32), (128, 1))
    MI16 = np.tile(np.arange(16, dtype=np.float32), (128, 1))
    REPM = np.zeros((16, 128), np.float32)
    for m in range(16):
        REPM[m, m::16] = 1.0
    in_maps = []
    for k in range(NCORES):
        esel = np.zeros((128, 2 * E), np.float32)
        esel[:, 2 * k] = 1.0
        esel[:, E + 2 * k + 1] = 1.0
        jsel = np.zeros((128, JB * J), np.float32)
        for i in range(JB):
            jsel[:, i * J + JB * k + i] = 1.0
        blk = slice(k * TB, (k + 1) * TB)
        in_maps.append({
            "hT_blk": np.ascontiguousarray(h[blk].T),
            "h_blk": np.ascontiguousarray(h[blk]),
            "h_bf": h_bf,
            "Wr": Wr,
            "W1b": np.ascontiguousarray(W1[2 * k:2 * k + 2]).astype(bf),
            "W2b": np.ascontiguousarray(W2[2 * k:2 * k + 2]).astype(bf),
            "TRI": TRI, "TRIX": TRIX, "ONE1": ONE1, "EIOTA": EIOTA,
            "ESEL": esel, "JSEL": jsel, "VALT": VALT, "CIO40": CIO40,
            "MI16": MI16, "REPM": REPM,
        })
    return in_maps


def get_program(local_only=False):
    key = "nc_local" if local_only else "nc"
    if key not in _cache:
        _cache[key] = _build_program(local_only)
    return _cache[key]


def kernel(h, Wr, W1, W2, topk, capacity, _return_results=False):
    assert int(topk) == TOPK and int(capacity) == C
    from concourse import bass_utils

    nc = get_program()
    in_maps = _prep_inputs(h, Wr, W1, W2)
    res = bass_utils.run_bass_kernel_spmd(nc, in_maps, core_ids=list(range(NCORES)))
    out = np.concatenate([res.results[k]["out"] for k in range(NCORES)], axis=0)
    if _return_results:
        return out, res
    return out
